# revision 20
# baseline (speedup 1.0000x reference)
"""Trainium2 Bass kernel for nn_Blender (per-style MLP blender).

Strategy
--------
Pure data parallel over the batch: each of the 8 NeuronCores processes
B/8 = 1024 samples with a full replica of the weights. No collectives.

On-chip layout is feature-major ([features -> partitions, batch -> free
dim]) so every GEMM contracts along the partition axis with batch as the
moving dim (N=512 = one fp32 PSUM bank). The host pre-transposes
global_styles to [S, D, B] and post-transposes the output back, so all
device DMA is contiguous. Output is written fp16 (adds ~5e-4
max-rel-err; tolerance is 2e-2).

fc GEMMs run in fp16 (1 cycle/row) accumulating into fp32 PSUM;
epilogues (bias/relu/residual) run on ACT/DVE. Every fc k-tile is a
full 128-row weight load so the PE preloads it into the background
weight buffer while the previous matmul streams (partial-row LDWEIGHTS
cannot be pulled ahead and stall the PE ~400ns each -- measured).

The bn1 stage reads a separate fp8(e4m3) copy of gs (bn_w1 prescaled
by 32 to clear the fp8 subnormal range; the h1 epilogue rescales by
1/32). The global-feature path carries ~5% of the output signal, so
fp8 there adds ~3e-3 max-rel-err. This halves the front-critical DMA:
phase 1 no longer shares tiles with the fc stage, which instead
streams its own fp16 gs tile per (style, chunk) right before use.

Algebraic folds (all exact):
  * bn2 + gm1 fuse: gm1_in = concat_s(h1_s @ bn_w2_s + bn_b2_s), so
    gm1_out = sum_s h1_s @ (bn_w2_s @ gm_w1_block_s) + folded bias.
    One block-stacked K=128 GEMM per 4-style group replaces bn2+gm1.
  * gm2 fold: fc1 consumes gmh (the gm hidden) directly through
    W_g' = gm_w2 @ fc_w1[:, :GH]; gm_b2's term goes into fc1's bias.
  * age rank-2: with age_b1 == 0 and ages >= 0 the age MLP is exactly
    affine in the scalar age: af = age*c + c0 (validated at runtime via
    lstsq; falls back to an explicit K=16 k-tile otherwise). Its fc1
    contribution is a rank-1 outer product age (x) d_s added into PSUM
    by the (otherwise idle) Vector engine, and a bias fold.

Schedule ("zipper", BC=1024 samples in 2 chunks of NB=512):
  16 junk matmuls on the first-landed tile -- holds the PE's HAM clock
    gate at 2.4 GHz through the DMA-paced front (cold PE runs 1.2 GHz)
  phase1(c0)                     -- front: 4.7 MB fp8 + consts
  fc(s,c0) for s=0..5            -- phase1(c1) loads issued after s=0,
                                    its matmuls emitted after s=4
  fc(s,c0), fc(s,c1) for s=6..17 -- one weight load serves both chunks
  fc(s,c1) for s=0..5            -- only these 6 styles re-load weights
This keeps every DMA window under ~250 GB/s (measured sustained load
rate is ~385 GB/s) while weights load 1.33x total instead of 2x for a
plain chunk-outer schedule.

Queue/ring assignment (a dma_start costs 0.65-2.4 us of *issue* time
on its engine, scaling with descriptor count -- the reason every gs
tile is host-pre-tiled to one contiguous 128 x 4 KB-descriptor DMA):
  sync:   per-style fc weights, fp16 gs tiles, consts
  scalar: fp8 gs tiles for phase 1 (idle at t=0 -> front loads start
          immediately; issued whole-group, 1 MB per DMA)
  gpsimd: style-0 weights (land during phase 1) + all out-stores, so
          the final drain only waits on the last output tiles
All fc biases live in one resident [128, S*8] tile (per-style 16 B-row
DMAs fragment the DMA stream).
"""

import ml_dtypes
import numpy as np

import concourse.bacc as bacc
import concourse.tile as tile
from concourse import mybir
from concourse.bass_utils import run_bass_kernel_spmd

S, D, BN, GH, AH, FCH = 18, 512, 32, 128, 16, 512
B = 8192
N_CORES = 8
BC = B // N_CORES          # samples per core
NB = 512                   # moving-dim (batch) tile = one fp32 PSUM bank
N_CHUNKS = BC // NB
GROUPS = [(0, 4), (4, 4), (8, 4), (12, 4), (16, 2)]
NG = len(GROUPS)
KT1 = 5                    # fc1 k-tiles: 4x gs(128) + gmh(128)
SOLO = 6                   # styles that run c0-only up front (c1 at the end)
PH1C1_AT = 4               # phase1(c1) MMs after this zipper entry
W1SC = 32.0                # bn_w1 fp8 prescale

F32 = mybir.dt.float32
F16 = mybir.dt.float16
F8 = mybir.dt.float8e4
MM_DT = mybir.dt.float16
NP_MM = np.float16
NP_F8 = ml_dtypes.float8_e4m3fn

_CACHE = {}


def build_program(rank2: bool = True):
    nc = bacc.Bacc("TRN2", target_bir_lowering=False, debug=False,
                   num_devices=N_CORES)
    mm = nc.tensor.matmul

    din = lambda name, shape, dt=MM_DT: nc.dram_tensor(name, shape, dt, kind="ExternalInput").ap()
    gsTd = din("gsTd", [S, N_CHUNKS, 128, 4 * NB])        # pre-tiled fp16
    gs8Td = din("gs8Td", [N_CHUNKS, 128, S * 4 * NB], F8)  # pre-tiled fp8
    bn_w1t8 = din("bn_w1t8", [128, S * 4 * BN], F8)
    bn_b1g = din("bn_b1g", [128, NG], F32)
    vg = din("vg", [128, NG * GH])
    gm_b1p = din("gm_b1p", [GH, 1], F32)
    fc_w1t = din("fc_w1t", [S, 128, KT1 * FCH])     # [s, p, kt*512 + h]
    fc_w2t = din("fc_w2t", [S, 128, 16 * 128])      # [s, p, (kt*4+dt)*128 + j]
    fc_bt = din("fc_bt", [128, S * 8], F32)         # [p, s*8 + (b1:0-3 | b2:4-7)]
    if rank2:
        ageB = din("ageB", [128, BC])               # age broadcast over partitions
        d_all = din("d_all", [128, S * 4], F32)     # rank-1 age dirs, [p, s*4+ht]
    else:
        afT = din("afT", [AH, BC])
        fa_w = din("fa_w", [AH, S * 4 * 128])       # af k-tile weights [a, (s,ht,j)]
    yT = nc.dram_tensor("yT", [S, D, BC], F16, kind="ExternalOutput").ap()

    Relu = mybir.ActivationFunctionType.Relu
    ADD = mybir.AluOpType.add
    MULT = mybir.AluOpType.mult

    with (
        tile.TileContext(nc) as tc,
        tc.tile_pool(name="consts", bufs=1) as consts,
        tc.tile_pool(name="gs8p", bufs=6) as gs8_pool,        # fp8 bn1 tiles
        tc.tile_pool(name="gstr", bufs=6) as gstr_pool,       # fp16 fc tiles
        tc.tile_pool(name="act1", bufs=3) as act1_pool,
        tc.tile_pool(name="wp", bufs=2) as w_pool,
        tc.tile_pool(name="y1p", bufs=2) as y1_pool,
        tc.tile_pool(name="outp", bufs=4) as out_pool,
        tc.tile_pool(name="ps", bufs=1, space="PSUM") as ps,
    ):
        # ---- per-style weight loads (see module docstring for queues) ----
        _wn = [0]
        def load_w(s, eng=None):
            _wn[0] += 1
            w1s = w_pool.tile([128, KT1 * FCH], MM_DT, tag="w1", bufs=4,
                              name=f"w1_{_wn[0]}_{s}")
            (eng or nc.sync).dma_start(w1s[:], fc_w1t[s, :, :])
            w2s = w_pool.tile([128, 16 * 128], MM_DT, tag="w2", bufs=4,
                              name=f"w2_{_wn[0]}_{s}")
            (eng or nc.sync).dma_start(w2s[:], fc_w2t[s, :, :])
            return w1s, w2s

        w0 = load_w(0, eng=nc.gpsimd)   # issued at t~0, lands during phase 1

        # ---- resident constants ----
        bn_w1_sb = consts.tile([128, S * 4 * BN], F8, tag="bn_w1")
        nc.sync.dma_start(bn_w1_sb[:], bn_w1t8[:])
        bn_b1_sb = consts.tile([128, NG], F32, tag="bn_b1")
        nc.sync.dma_start(bn_b1_sb[:], bn_b1g[:])
        vg_sb = consts.tile([128, NG * GH], MM_DT, tag="vg")
        nc.sync.dma_start(vg_sb[:], vg[:])
        gm_b1_sb = consts.tile([GH, 1], F32, tag="gm_b1")
        nc.sync.dma_start(gm_b1_sb[:], gm_b1p[:])
        fcb_sb = consts.tile([128, S * 8], F32, tag="fc_bt")
        if rank2:
            age_sb = consts.tile([128, BC], MM_DT, tag="ageB")
            d_sb = consts.tile([128, S * 4], F32, tag="d_all")
        else:
            af_sb = consts.tile([AH, BC], MM_DT, tag="af")
            fa_sb = consts.tile([AH, S * 4 * 128], MM_DT, tag="fa_w")

        def load_fc_consts():        # needed only from fc(s0) on (~30us)
            nc.sync.dma_start(fcb_sb[:], fc_bt[:])
            if rank2:
                nc.sync.dma_start(age_sb[:], ageB[:])
                nc.sync.dma_start(d_sb[:], d_all[:])
            else:
                nc.sync.dma_start(af_sb[:], afT[:])
                nc.sync.dma_start(fa_sb[:], fa_w[:])
        gmh_sb = [consts.tile([GH, NB], MM_DT, tag=f"gmh{c}", name=f"gmh{c}")
                  for c in range(N_CHUNKS)]

        def load_gs16(s, c):
            t = gstr_pool.tile([128, 4 * NB], MM_DT, tag="g16",
                               name=f"gs_{s}_{c}")
            nc.sync.dma_start(t[:], gsTd[s, c])
            return t

        def phase1_loads(c):
            tiles = []
            for gi, (s0, ng) in enumerate(GROUPS):
                t8 = gs8_pool.tile([128, ng * 4 * NB], F8, tag="g8",
                                   name=f"gs8_{gi}_{c}")
                nc.scalar.dma_start(
                    t8[:], gs8Td[c, :, s0 * 4 * NB:(s0 + ng) * 4 * NB])
                tiles.append(t8)
            return tiles

        def phase1(c, tiles, mid_hook=None):
            hook_out = None
            ps_g1 = ps.tile([GH, NB], F32, tag="g1", bufs=1, name=f"ps_g1_{c}")
            for gi, (s0, ng) in enumerate(GROUPS):
                pN = 32 * ng
                t8 = tiles[gi]
                ps_h1 = ps.tile([128, NB], F32, tag="y1", bufs=3,
                                name=f"ps_h1_{gi}_{c}")
                for kt in range(4):         # kt-major: col-groups concurrent
                    for j in range(ng):
                        s = s0 + j
                        mm(ps_h1[32 * j:32 * j + 32, :],
                           bn_w1_sb[:, (s * 4 + kt) * BN:(s * 4 + kt + 1) * BN],
                           t8[:, (j * 4 + kt) * NB:(j * 4 + kt + 1) * NB],
                           start=(kt == 0), stop=(kt == 3),
                           tile_position=(0, 32 * j))
                h1 = act1_pool.tile([128, NB], MM_DT, tag="h1s", name=f"h1_{gi}_{c}")
                nc.scalar.activation(h1[:pN, :], ps_h1[:pN, :], Relu,
                                     bias=bn_b1_sb[:pN, gi:gi + 1],
                                     scale=1.0 / W1SC)
                mm(ps_g1[:], vg_sb[:pN, gi * GH:(gi + 1) * GH], h1[:pN, :],
                   start=(gi == 0), stop=(gi == NG - 1))
                if gi == 2 and mid_hook is not None:
                    hook_out = mid_hook()
            nc.scalar.activation(gmh_sb[c][:], ps_g1[:], Relu, bias=gm_b1_sb[:])
            return hook_out

        def fc1_open(s, c, w1s, gs_sb, tag="y1", bufs=3):
            opens = []
            for ht in range(4):
                h0 = ht * 128
                ps_y1 = ps.tile([128, NB], F32, tag=tag, bufs=bufs,
                                name=f"ps_y1_{s}_{c}_{ht}")
                for kt in range(4):      # gs k-tiles (no gmh dep)
                    mm(ps_y1[:],
                       w1s[:, kt * FCH + h0:kt * FCH + h0 + 128],
                       gs_sb[:, kt * NB:(kt + 1) * NB],
                       start=(kt == 0), stop=False)
                opens.append(ps_y1)
            return opens

        def fc_close(s, c, w1s, w2s, gs_sb, opens):
            b0 = c * NB
            y1 = []
            for ht in range(4):
                h0 = ht * 128
                ps_y1 = opens[ht]
                if not rank2:
                    mm(ps_y1[:],         # af k-tile (K=16)
                       fa_sb[:, (s * 4 + ht) * 128:(s * 4 + ht + 1) * 128],
                       af_sb[:, b0:b0 + NB],
                       start=False, stop=False)
                mm(ps_y1[:],             # gmh k-tile last
                   w1s[:, 4 * FCH + h0:4 * FCH + h0 + 128],
                   gmh_sb[c][:],
                   start=False, stop=True)
                if rank2:                # += age (x) d_s,ht  (rank-1, DVE)
                    nc.vector.scalar_tensor_tensor(
                        ps_y1[:], age_sb[:, b0:b0 + NB],
                        d_sb[:, s * 4 + ht:s * 4 + ht + 1],
                        ps_y1[:], op0=MULT, op1=ADD)
                y1t = y1_pool.tile([128, NB], MM_DT, tag=f"y1_{ht}",
                                   name=f"y1_{s}_{c}_{ht}")
                nc.scalar.activation(y1t[:], ps_y1[:], Relu,
                                     bias=fcb_sb[:, s * 8 + ht:s * 8 + ht + 1])
                y1.append(y1t)
            for dt_ in range(4):
                ps_y = ps.tile([128, NB], F32, tag="y", bufs=4,
                               name=f"ps_y_{s}_{c}_{dt_}")
                for kt in range(4):
                    mm(ps_y[:],
                       w2s[:, (kt * 4 + dt_) * 128:(kt * 4 + dt_ + 1) * 128],
                       y1[kt][:],
                       start=(kt == 0), stop=(kt == 3))
                o = out_pool.tile([128, NB], F16, tag="o", name=f"o_{s}_{c}_{dt_}")
                nc.vector.scalar_tensor_tensor(
                    o[:], ps_y[:], fcb_sb[:, s * 8 + 4 + dt_:s * 8 + 5 + dt_],
                    gs_sb[:, dt_ * NB:(dt_ + 1) * NB], op0=ADD, op1=ADD)
                nc.gpsimd.dma_start(yT[s, dt_ * 128:(dt_ + 1) * 128, b0:b0 + NB], o[:])

        def fc_style(s, c, w1s, w2s, gs_sb):
            fc_close(s, c, w1s, w2s, gs_sb,
                     fc1_open(s, c, w1s, gs_sb))

        # ---------------- zipper schedule ----------------
        # HAM warm-up: the PE clock sits at 1.2 GHz until ~3.4us of
        # sustained matmul activity. Fill the front DMA wait with junk
        # matmuls on the first-loaded tile so phase 1 runs at 2.4 GHz.
        for wi in range(16):
            ps_warm = ps.tile([128, NB], F32, tag="y", bufs=4,
                              name=f"warm_{wi}")
            mm(ps_warm[:], bn_w1_sb[:, :128], bn_w1_sb[:, :NB],
               start=True, stop=True)
        phase1(0, phase1_loads(0))
        load_fc_consts()
        entries = ([(s, [0]) for s in range(SOLO)]
                   + [(s, [0, 1]) for s in range(SOLO, S)]
                   + [(s, [1]) for s in range(SOLO)])
        calls = [(s, c) for s, chs in entries for c in chs]
        gs16 = lambda sc: load_gs16(sc[0], sc[1])
        gs_q = [gs16(calls[0]), gs16(calls[1]), gs16(calls[2])]
        w_q = [w0, load_w(entries[1][0]), load_w(entries[2][0])]
        ci = 0
        p1c1_tiles = None
        for ei, (s, chs) in enumerate(entries):
            if ei == 1:
                p1c1_tiles = phase1_loads(1)
            if ei + 3 < len(entries):
                w_q.append(load_w(entries[ei + 3][0]))
            w1s, w2s = w_q.pop(0)
            for c in chs:
                if ci + 3 < len(calls):
                    gs_q.append(gs16(calls[ci + 3]))
                fc_style(s, c, w1s, w2s, gs_q.pop(0))
                ci += 1
            if ei == PH1C1_AT:
                phase1(1, p1c1_tiles)

    nc.compile()
    return nc


def _prep_weights(bn_w1, bn_b1, bn_w2, bn_b2, gm_w1, gm_b1, gm_w2, gm_b2,
                  fc_w1, fc_b1, fc_w2, fc_b2, c_age, c0_age, rank2):
    f = np.float32
    h = NP_MM
    # [p, (s, kt, j)] : bn_w1[s, kt*128+p, j], prescaled x32 for fp8
    bn_w1t8 = np.ascontiguousarray(
        (bn_w1.astype(f) * W1SC).reshape(S, 4, 128, BN)
        .transpose(2, 0, 1, 3).reshape(128, S * 4 * BN)).astype(NP_F8)
    bn_b1g = np.zeros((128, NG), f)
    for gi, (s0, ng) in enumerate(GROUPS):
        for j in range(ng):
            bn_b1g[32 * j:32 * j + 32, gi] = bn_b1[s0 + j]
    # fused bn2 @ gm_w1-block, group-stacked: vg[32j:+32, gi*128:+128] = V[s0+j]
    gm_w1b = gm_w1.reshape(S, BN, GH).astype(f)
    V = np.einsum('skm,smg->skg', bn_w2.astype(f), gm_w1b)      # [S, 32, 128]
    vg = np.zeros((128, NG * GH), h)
    for gi, (s0, ng) in enumerate(GROUPS):
        for j in range(ng):
            vg[32 * j:32 * j + 32, gi * GH:(gi + 1) * GH] = V[s0 + j]
    gm_b1p = (gm_b1.astype(f)
              + np.einsum('sm,smg->g', bn_b2.astype(f), gm_w1b)).reshape(GH, 1)

    W_g = fc_w1[:, :GH, :].astype(f)                            # [S, 128, 512]
    W_a = fc_w1[:, GH:GH + AH, :].astype(f)                     # [S, 16, 512]
    # fc1 rows: [gs (4x128) | gmh (gm_w2 @ W_g)]
    w1p = np.empty((S, KT1 * 128, FCH), f)
    w1p[:, :4 * 128] = fc_w1[:, GH + AH:]
    w1p[:, 4 * 128:] = np.einsum('kj,sjf->skf', gm_w2.astype(f), W_g)
    fc_w1t = np.ascontiguousarray(
        w1p.reshape(S, KT1, 128, FCH).transpose(0, 2, 1, 3).reshape(S, 128, KT1 * FCH), h)
    b1_full = (fc_b1.astype(f)
               + np.einsum('j,sjf->sf', gm_b2.astype(f), W_g))
    if rank2:
        b1_full = b1_full + np.einsum('a,saf->sf', c0_age, W_a)
    # biases combined: fc_bt[p, s*8 + t] = b1[s, t*128+p] (t<4) | b2[s, (t-4)*128+p]
    fc_bt = np.empty((128, S * 8), f)
    fc_bt.reshape(128, S, 8)[:, :, :4] = b1_full.reshape(S, 4, 128).transpose(2, 0, 1)
    fc_bt.reshape(128, S, 8)[:, :, 4:] = fc_b2.astype(f).reshape(S, 4, 128).transpose(2, 0, 1)
    fc_w2t = np.ascontiguousarray(
        fc_w2.reshape(S, 4, 128, 4, 128).transpose(0, 2, 1, 3, 4).reshape(S, 128, 16 * 128), h)
    out = dict(bn_w1t8=bn_w1t8, bn_b1g=bn_b1g, vg=vg, gm_b1p=gm_b1p,
               fc_w1t=fc_w1t, fc_w2t=fc_w2t, fc_bt=fc_bt)
    if rank2:
        d = np.einsum('a,saf->sf', c_age, W_a)                  # [S, 512]
        out["d_all"] = np.ascontiguousarray(
            d.reshape(S, 4, 128).transpose(2, 0, 1).reshape(128, S * 4), f)
    else:
        # af k-tile weights: fa_w[a, (s*4+ht)*128 + j] = W_a[s, a, ht*128+j]
        out["fa_w"] = np.ascontiguousarray(
            W_a.reshape(S, AH, 4, 128).transpose(1, 0, 2, 3).reshape(AH, S * 4 * 128), h)
    return out


def run(inputs: dict, trace: bool = False):
    """Build in_maps from full inputs, run SPMD on 8 cores, return
    (full_output, BassKernelResults)."""
    gs = inputs["global_styles"]
    ages = np.asarray(inputs["target_ages"], np.float32)
    # host: exact fp32 age MLP (tiny), then affine-in-age fit
    af = np.maximum(ages[:, None] @ inputs["age_w1"] + inputs["age_b1"], 0.0)
    af = (af @ inputs["age_w2"] + inputs["age_b2"]).astype(np.float32)  # [B, 16]
    A = np.stack([ages, np.ones_like(ages)], axis=1)                    # [B, 2]
    sol, *_ = np.linalg.lstsq(A.astype(np.float64), af.astype(np.float64),
                              rcond=None)
    resid = np.abs(af - (A @ sol.astype(np.float32))).max()
    rank2 = bool(resid <= 1e-4 * max(1.0, np.abs(af).max()))
    c_age, c0_age = sol[0].astype(np.float32), sol[1].astype(np.float32)

    key = ("nc", rank2)
    if key not in _CACHE:
        _CACHE[key] = build_program(rank2)
    nc = _CACHE[key]

    w = _prep_weights(
        inputs["bn_w1"], inputs["bn_b1"], inputs["bn_w2"], inputs["bn_b2"],
        inputs["gm_w1"], inputs["gm_b1"], inputs["gm_w2"], inputs["gm_b2"],
        inputs["fc_w1"], inputs["fc_b1"], inputs["fc_w2"], inputs["fc_b2"],
        c_age, c0_age, rank2)

    gsT_full = np.ascontiguousarray(gs.transpose(1, 2, 0).astype(NP_MM))  # [S, D, B]
    ages16 = ages.astype(NP_MM)
    afT_full = np.ascontiguousarray(af.T.astype(NP_MM))                   # [16, B]
    in_maps = []
    for c in range(N_CORES):
        sl = slice(c * BC, (c + 1) * BC)
        m = dict(w)
        # tile-contiguous layouts: one DMA per tile = 128 fat descriptors
        # (strided gs DMAs cost 1.4-2.4us of issue time each -- measured)
        gsc = gsT_full[:, :, sl].reshape(S, 4, 128, N_CHUNKS, NB)
        m["gsTd"] = np.ascontiguousarray(
            gsc.transpose(0, 3, 2, 1, 4).reshape(S, N_CHUNKS, 128, 4 * NB))
        m["gs8Td"] = np.ascontiguousarray(
            gsc.transpose(3, 2, 0, 1, 4).reshape(N_CHUNKS, 128, S * 4 * NB)
            .astype(NP_F8))
        if rank2:
            m["ageB"] = np.ascontiguousarray(
                np.broadcast_to(ages16[None, sl], (128, BC)))
        else:
            m["afT"] = np.ascontiguousarray(afT_full[:, sl])
        in_maps.append(m)

    res = run_bass_kernel_spmd(nc, in_maps, core_ids=list(range(N_CORES)),
                               trace=trace)
    yT = np.concatenate([res.results[c]["yT"][:, :, :] for c in range(N_CORES)],
                        axis=2)                              # [S, D, B] fp16
    y = yT.astype(np.float32).transpose(2, 0, 1)             # [B, S, D]
    return np.ascontiguousarray(y), res


def kernel(**inputs) -> np.ndarray:
    y, _ = run(inputs, trace=False)
    return y


# revision 21
# speedup vs baseline: 1.1915x; 1.1915x over previous
"""Trainium2 Bass kernel for nn_Blender (per-style MLP blender).

Strategy
--------
Pure data parallel over the batch: each of the 8 NeuronCores processes
B/8 = 1024 samples with a full replica of the weights. No collectives.

On-chip layout is feature-major ([features -> partitions, batch -> free
dim]) so every GEMM contracts along the partition axis with batch as the
moving dim (N=512 = one fp32 PSUM bank). The host pre-transposes
global_styles to [S, D, B] and post-transposes the output back, so all
device DMA is contiguous. Output is written fp16 (adds ~5e-4
max-rel-err; tolerance is 2e-2).

fc GEMMs run in fp16 (1 cycle/row) accumulating into fp32 PSUM;
epilogues (bias/relu/residual) run on ACT/DVE. Every fc k-tile is a
full 128-row weight load so the PE preloads it into the background
weight buffer while the previous matmul streams (partial-row LDWEIGHTS
cannot be pulled ahead and stall the PE ~400ns each -- measured).

The bn1 stage reads a separate fp8(e4m3) copy of gs (bn_w1 prescaled
by 32 to clear the fp8 subnormal range; the h1 epilogue rescales by
1/32). The global-feature path carries ~5% of the output signal, so
fp8 there adds ~3e-3 max-rel-err. This halves the front-critical DMA:
phase 1 no longer shares tiles with the fc stage, which instead
streams its own fp16 gs tile per (style, chunk) right before use.

Algebraic folds (all exact):
  * bn2 + gm1 fuse: gm1_in = concat_s(h1_s @ bn_w2_s + bn_b2_s), so
    gm1_out = sum_s h1_s @ (bn_w2_s @ gm_w1_block_s) + folded bias.
    One block-stacked K=128 GEMM per 4-style group replaces bn2+gm1.
  * gm2 fold: fc1 consumes gmh (the gm hidden) directly through
    W_g' = gm_w2 @ fc_w1[:, :GH]; gm_b2's term goes into fc1's bias.
  * age rank-2: with age_b1 == 0 and ages >= 0 the age MLP is exactly
    affine in the scalar age: af = age*c + c0 (validated at runtime via
    lstsq; falls back to an explicit K=16 k-tile otherwise). Its fc1
    contribution is a rank-1 outer product age (x) d_s added into PSUM
    by the (otherwise idle) Vector engine, and a bias fold.

Schedule ("zipper", BC=1024 samples in 2 chunks of NB=512):
  16 junk matmuls on the first-landed tile -- holds the PE's HAM clock
    gate at 2.4 GHz through the DMA-paced front (cold PE runs 1.2 GHz)
  phase1(c0)                     -- front: 4.7 MB fp8 + consts
  fc(s,c0) for s=0..5            -- phase1(c1) loads issued after s=0,
                                    its matmuls emitted after s=4
  fc(s,c0), fc(s,c1) for s=6..17 -- one weight load serves both chunks
  fc(s,c1) for s=0..5            -- only these 6 styles re-load weights
This keeps every DMA window under ~250 GB/s (measured sustained load
rate is ~385 GB/s) while weights load 1.33x total instead of 2x for a
plain chunk-outer schedule.

Queue/ring assignment (a dma_start costs 0.65-2.4 us of *issue* time
on its engine, scaling with descriptor count -- the reason every gs
tile is host-pre-tiled to one contiguous 128 x 4 KB-descriptor DMA):
  sync:   per-style fc weights, fp16 gs tiles, consts
  scalar: fp8 gs tiles for phase 1 (idle at t=0 -> front loads start
          immediately; issued whole-group, 1 MB per DMA)
  gpsimd: style-0 weights (land during phase 1) + all out-stores, so
          the final drain only waits on the last output tiles
All fc biases live in one resident [128, S*8] tile (per-style 16 B-row
DMAs fragment the DMA stream).
"""

import ml_dtypes
import numpy as np

import concourse.bacc as bacc
import concourse.tile as tile
from concourse import mybir
from concourse.bass_utils import run_bass_kernel_spmd

S, D, BN, GH, AH, FCH = 18, 512, 32, 128, 16, 512
B = 8192
N_CORES = 8
BC = B // N_CORES          # samples per core
NB = 512                   # moving-dim (batch) tile = one fp32 PSUM bank
N_CHUNKS = BC // NB
GROUPS = [(0, 4), (4, 4), (8, 4), (12, 4), (16, 2)]
NG = len(GROUPS)
KT1 = 5                    # fc1 k-tiles: 4x gs(128) + gmh(128)
SOLO = 6                   # styles that run c0-only up front (c1 at the end)
PH1C1_AT = 4               # phase1(c1) MMs after this zipper entry
W1SC = 32.0                # bn_w1 fp8 prescale

F32 = mybir.dt.float32
F16 = mybir.dt.float16
F8 = mybir.dt.float8e4
MM_DT = mybir.dt.float16
NP_MM = np.float16
NP_F8 = ml_dtypes.float8_e4m3fn

_CACHE = {}


def build_program(rank2: bool = True):
    nc = bacc.Bacc("TRN2", target_bir_lowering=False, debug=False,
                   num_devices=N_CORES)
    mm = nc.tensor.matmul

    din = lambda name, shape, dt=MM_DT: nc.dram_tensor(name, shape, dt, kind="ExternalInput").ap()
    gsTd = din("gsTd", [S, N_CHUNKS, 128, 4 * NB])        # pre-tiled fp16
    gs8Td = din("gs8Td", [N_CHUNKS, 128, S * 4 * NB], F8)  # pre-tiled fp8
    bn_w1t8 = din("bn_w1t8", [128, S * 4 * BN], F8)
    bn_b1g = din("bn_b1g", [128, NG], F32)
    vg = din("vg", [128, NG * GH])
    gm_b1p = din("gm_b1p", [GH, 1], F32)
    fc_w1t = din("fc_w1t", [S, 128, KT1 * FCH])     # [s, p, kt*512 + h]
    fc_w2t = din("fc_w2t", [S, 128, 16 * 128])      # [s, p, (kt*4+dt)*128 + j]
    fc_bt = din("fc_bt", [128, S * 8], F32)         # [p, s*8 + (b1:0-3 | b2:4-7)]
    if rank2:
        ageB = din("ageB", [128, BC])               # age broadcast over partitions
        d_all = din("d_all", [128, S * 4], F32)     # rank-1 age dirs, [p, s*4+ht]
    else:
        afT = din("afT", [AH, BC])
        fa_w = din("fa_w", [AH, S * 4 * 128])       # af k-tile weights [a, (s,ht,j)]
    yT = nc.dram_tensor("yT", [S, D, BC], F16, kind="ExternalOutput").ap()

    Relu = mybir.ActivationFunctionType.Relu
    ADD = mybir.AluOpType.add
    MULT = mybir.AluOpType.mult

    with (
        tile.TileContext(nc) as tc,
        tc.tile_pool(name="consts", bufs=1) as consts,
        tc.tile_pool(name="gs8p", bufs=6) as gs8_pool,        # fp8 bn1 tiles
        tc.tile_pool(name="gstr", bufs=4) as gstr_pool,       # fp16 fc tiles
        tc.tile_pool(name="act1", bufs=3) as act1_pool,
        tc.tile_pool(name="wp", bufs=2) as w_pool,
        tc.tile_pool(name="y1p", bufs=2) as y1_pool,
        tc.tile_pool(name="outp", bufs=4) as out_pool,
        tc.tile_pool(name="ps", bufs=1, space="PSUM") as ps,
    ):
        # ---- per-style weight loads (see module docstring for queues) ----
        _wn = [0]
        def load_w(s, eng=None):
            _wn[0] += 1
            w1s = w_pool.tile([128, KT1 * FCH], MM_DT, tag="w1", bufs=3,
                              name=f"w1_{_wn[0]}_{s}")
            (eng or nc.sync).dma_start(w1s[:], fc_w1t[s, :, :])
            w2s = w_pool.tile([128, 16 * 128], MM_DT, tag="w2", bufs=3,
                              name=f"w2_{_wn[0]}_{s}")
            (eng or nc.sync).dma_start(w2s[:], fc_w2t[s, :, :])
            return w1s, w2s

        w0 = load_w(0, eng=nc.gpsimd)   # issued at t~0, lands during phase 1

        # ---- resident constants ----
        bn_w1_sb = consts.tile([128, S * 4 * BN], F8, tag="bn_w1")
        nc.sync.dma_start(bn_w1_sb[:], bn_w1t8[:])
        bn_b1_sb = consts.tile([128, NG], F32, tag="bn_b1")
        nc.sync.dma_start(bn_b1_sb[:], bn_b1g[:])
        vg_sb = consts.tile([128, NG * GH], MM_DT, tag="vg")
        nc.sync.dma_start(vg_sb[:], vg[:])
        gm_b1_sb = consts.tile([GH, 1], F32, tag="gm_b1")
        nc.sync.dma_start(gm_b1_sb[:], gm_b1p[:])
        fcb_sb = consts.tile([128, S * 8], F32, tag="fc_bt")
        if rank2:
            age_sb = consts.tile([128, BC], MM_DT, tag="ageB")
            d_sb = consts.tile([128, S * 4], F32, tag="d_all")
        else:
            af_sb = consts.tile([AH, BC], MM_DT, tag="af")
            fa_sb = consts.tile([AH, S * 4 * 128], MM_DT, tag="fa_w")

        def load_fc_consts():        # needed only from fc(s0) on (~30us)
            nc.sync.dma_start(fcb_sb[:], fc_bt[:])
            if rank2:
                nc.sync.dma_start(age_sb[:], ageB[:])
                nc.sync.dma_start(d_sb[:], d_all[:])
            else:
                nc.sync.dma_start(af_sb[:], afT[:])
                nc.sync.dma_start(fa_sb[:], fa_w[:])
        gmh_sb = [consts.tile([GH, NB], MM_DT, tag=f"gmh{c}", name=f"gmh{c}")
                  for c in range(N_CHUNKS)]

        def load_gs16(s, c):
            t = gstr_pool.tile([128, 4 * NB], MM_DT, tag="g16",
                               name=f"gs_{s}_{c}")
            nc.sync.dma_start(t[:], gsTd[s, c])
            return t

        def phase1_loads(c):
            tiles = []
            for gi, (s0, ng) in enumerate(GROUPS):
                t8 = gs8_pool.tile([128, ng * 4 * NB], F8, tag="g8",
                                   name=f"gs8_{gi}_{c}")
                nc.scalar.dma_start(
                    t8[:], gs8Td[c, :, s0 * 4 * NB:(s0 + ng) * 4 * NB])
                tiles.append(t8)
            return tiles

        def phase1(c, tiles, mid_hook=None):
            hook_out = None
            ps_g1 = ps.tile([GH, NB], F32, tag="g1", bufs=1, name=f"ps_g1_{c}")
            for gi, (s0, ng) in enumerate(GROUPS):
                pN = 32 * ng
                t8 = tiles[gi]
                ps_h1 = ps.tile([128, NB], F32, tag="y1", bufs=3,
                                name=f"ps_h1_{gi}_{c}")
                for kt in range(4):         # kt-major: col-groups concurrent
                    for j in range(ng):
                        s = s0 + j
                        mm(ps_h1[32 * j:32 * j + 32, :],
                           bn_w1_sb[:, (s * 4 + kt) * BN:(s * 4 + kt + 1) * BN],
                           t8[:, (j * 4 + kt) * NB:(j * 4 + kt + 1) * NB],
                           start=(kt == 0), stop=(kt == 3),
                           tile_position=(0, 32 * j))
                h1 = act1_pool.tile([128, NB], MM_DT, tag="h1s", name=f"h1_{gi}_{c}")
                nc.scalar.activation(h1[:pN, :], ps_h1[:pN, :], Relu,
                                     bias=bn_b1_sb[:pN, gi:gi + 1],
                                     scale=1.0 / W1SC)
                mm(ps_g1[:], vg_sb[:pN, gi * GH:(gi + 1) * GH], h1[:pN, :],
                   start=(gi == 0), stop=(gi == NG - 1))
                if gi == 2 and mid_hook is not None:
                    hook_out = mid_hook()
            nc.scalar.activation(gmh_sb[c][:], ps_g1[:], Relu, bias=gm_b1_sb[:])
            return hook_out

        def fc1_open(s, c, w1s, gs_sb, tag="y1", bufs=3):
            opens = []
            for ht in range(4):
                h0 = ht * 128
                ps_y1 = ps.tile([128, NB], F32, tag=tag, bufs=bufs,
                                name=f"ps_y1_{s}_{c}_{ht}")
                for kt in range(4):      # gs k-tiles (no gmh dep)
                    mm(ps_y1[:],
                       w1s[:, kt * FCH + h0:kt * FCH + h0 + 128],
                       gs_sb[:, kt * NB:(kt + 1) * NB],
                       start=(kt == 0), stop=False)
                opens.append(ps_y1)
            return opens

        def fc_close(s, c, w1s, w2s, gs_sb, opens):
            b0 = c * NB
            y1 = []
            for ht in range(4):
                h0 = ht * 128
                ps_y1 = opens[ht]
                if not rank2:
                    mm(ps_y1[:],         # af k-tile (K=16)
                       fa_sb[:, (s * 4 + ht) * 128:(s * 4 + ht + 1) * 128],
                       af_sb[:, b0:b0 + NB],
                       start=False, stop=False)
                mm(ps_y1[:],             # gmh k-tile last
                   w1s[:, 4 * FCH + h0:4 * FCH + h0 + 128],
                   gmh_sb[c][:],
                   start=False, stop=True)
                if rank2:                # += age (x) d_s,ht  (rank-1, DVE)
                    nc.vector.scalar_tensor_tensor(
                        ps_y1[:], age_sb[:, b0:b0 + NB],
                        d_sb[:, s * 4 + ht:s * 4 + ht + 1],
                        ps_y1[:], op0=MULT, op1=ADD)
                y1t = y1_pool.tile([128, NB], MM_DT, tag=f"y1_{ht}",
                                   name=f"y1_{s}_{c}_{ht}")
                nc.scalar.activation(y1t[:], ps_y1[:], Relu,
                                     bias=fcb_sb[:, s * 8 + ht:s * 8 + ht + 1])
                y1.append(y1t)
            for dt_ in range(4):
                ps_y = ps.tile([128, NB], F32, tag="y", bufs=4,
                               name=f"ps_y_{s}_{c}_{dt_}")
                for kt in range(4):
                    mm(ps_y[:],
                       w2s[:, (kt * 4 + dt_) * 128:(kt * 4 + dt_ + 1) * 128],
                       y1[kt][:],
                       start=(kt == 0), stop=(kt == 3))
                o = out_pool.tile([128, NB], F16, tag="o", name=f"o_{s}_{c}_{dt_}")
                nc.vector.scalar_tensor_tensor(
                    o[:], ps_y[:], fcb_sb[:, s * 8 + 4 + dt_:s * 8 + 5 + dt_],
                    gs_sb[:, dt_ * NB:(dt_ + 1) * NB], op0=ADD, op1=ADD)
                nc.gpsimd.dma_start(yT[s, dt_ * 128:(dt_ + 1) * 128, b0:b0 + NB], o[:])

        def fc_style(s, c, w1s, w2s, gs_sb):
            fc_close(s, c, w1s, w2s, gs_sb,
                     fc1_open(s, c, w1s, gs_sb))

        # ---------------- zipper schedule ----------------
        # HAM warm-up: the PE clock sits at 1.2 GHz until ~3.4us of
        # sustained matmul activity. Fill the front DMA wait with junk
        # matmuls on the first-loaded tile so phase 1 runs at 2.4 GHz.
        for wi in range(16):
            ps_warm = ps.tile([128, NB], F32, tag="y", bufs=4,
                              name=f"warm_{wi}")
            mm(ps_warm[:], bn_w1_sb[:, :128], bn_w1_sb[:, :NB],
               start=True, stop=True)
        phase1(0, phase1_loads(0))
        load_fc_consts()
        entries = ([(s, [0]) for s in range(SOLO)]
                   + [(s, [0, 1]) for s in range(SOLO, S)]
                   + [(s, [1]) for s in range(SOLO)])
        calls = [(s, c) for s, chs in entries for c in chs]
        gs16 = lambda sc: load_gs16(sc[0], sc[1])
        gs_q = [gs16(calls[0]), gs16(calls[1])]
        w_q = [w0, load_w(entries[1][0])]
        ci = 0
        p1c1_tiles = None
        for ei, (s, chs) in enumerate(entries):
            if ei == 1:
                p1c1_tiles = phase1_loads(1)
            if ei + 2 < len(entries):
                w_q.append(load_w(entries[ei + 2][0]))
            w1s, w2s = w_q.pop(0)
            for c in chs:
                if ci + 2 < len(calls):
                    gs_q.append(gs16(calls[ci + 2]))
                fc_style(s, c, w1s, w2s, gs_q.pop(0))
                ci += 1
            if ei == PH1C1_AT:
                phase1(1, p1c1_tiles)

    nc.compile()
    return nc


def _prep_weights(bn_w1, bn_b1, bn_w2, bn_b2, gm_w1, gm_b1, gm_w2, gm_b2,
                  fc_w1, fc_b1, fc_w2, fc_b2, c_age, c0_age, rank2):
    f = np.float32
    h = NP_MM
    # [p, (s, kt, j)] : bn_w1[s, kt*128+p, j], prescaled x32 for fp8
    bn_w1t8 = np.ascontiguousarray(
        (bn_w1.astype(f) * W1SC).reshape(S, 4, 128, BN)
        .transpose(2, 0, 1, 3).reshape(128, S * 4 * BN)).astype(NP_F8)
    bn_b1g = np.zeros((128, NG), f)
    for gi, (s0, ng) in enumerate(GROUPS):
        for j in range(ng):
            bn_b1g[32 * j:32 * j + 32, gi] = bn_b1[s0 + j]
    # fused bn2 @ gm_w1-block, group-stacked: vg[32j:+32, gi*128:+128] = V[s0+j]
    gm_w1b = gm_w1.reshape(S, BN, GH).astype(f)
    V = np.einsum('skm,smg->skg', bn_w2.astype(f), gm_w1b)      # [S, 32, 128]
    vg = np.zeros((128, NG * GH), h)
    for gi, (s0, ng) in enumerate(GROUPS):
        for j in range(ng):
            vg[32 * j:32 * j + 32, gi * GH:(gi + 1) * GH] = V[s0 + j]
    gm_b1p = (gm_b1.astype(f)
              + np.einsum('sm,smg->g', bn_b2.astype(f), gm_w1b)).reshape(GH, 1)

    W_g = fc_w1[:, :GH, :].astype(f)                            # [S, 128, 512]
    W_a = fc_w1[:, GH:GH + AH, :].astype(f)                     # [S, 16, 512]
    # fc1 rows: [gs (4x128) | gmh (gm_w2 @ W_g)]
    w1p = np.empty((S, KT1 * 128, FCH), f)
    w1p[:, :4 * 128] = fc_w1[:, GH + AH:]
    w1p[:, 4 * 128:] = np.einsum('kj,sjf->skf', gm_w2.astype(f), W_g)
    fc_w1t = np.ascontiguousarray(
        w1p.reshape(S, KT1, 128, FCH).transpose(0, 2, 1, 3).reshape(S, 128, KT1 * FCH), h)
    b1_full = (fc_b1.astype(f)
               + np.einsum('j,sjf->sf', gm_b2.astype(f), W_g))
    if rank2:
        b1_full = b1_full + np.einsum('a,saf->sf', c0_age, W_a)
    # biases combined: fc_bt[p, s*8 + t] = b1[s, t*128+p] (t<4) | b2[s, (t-4)*128+p]
    fc_bt = np.empty((128, S * 8), f)
    fc_bt.reshape(128, S, 8)[:, :, :4] = b1_full.reshape(S, 4, 128).transpose(2, 0, 1)
    fc_bt.reshape(128, S, 8)[:, :, 4:] = fc_b2.astype(f).reshape(S, 4, 128).transpose(2, 0, 1)
    fc_w2t = np.ascontiguousarray(
        fc_w2.reshape(S, 4, 128, 4, 128).transpose(0, 2, 1, 3, 4).reshape(S, 128, 16 * 128), h)
    out = dict(bn_w1t8=bn_w1t8, bn_b1g=bn_b1g, vg=vg, gm_b1p=gm_b1p,
               fc_w1t=fc_w1t, fc_w2t=fc_w2t, fc_bt=fc_bt)
    if rank2:
        d = np.einsum('a,saf->sf', c_age, W_a)                  # [S, 512]
        out["d_all"] = np.ascontiguousarray(
            d.reshape(S, 4, 128).transpose(2, 0, 1).reshape(128, S * 4), f)
    else:
        # af k-tile weights: fa_w[a, (s*4+ht)*128 + j] = W_a[s, a, ht*128+j]
        out["fa_w"] = np.ascontiguousarray(
            W_a.reshape(S, AH, 4, 128).transpose(1, 0, 2, 3).reshape(AH, S * 4 * 128), h)
    return out


def run(inputs: dict, trace: bool = False):
    """Build in_maps from full inputs, run SPMD on 8 cores, return
    (full_output, BassKernelResults)."""
    gs = inputs["global_styles"]
    ages = np.asarray(inputs["target_ages"], np.float32)
    # host: exact fp32 age MLP (tiny), then affine-in-age fit
    af = np.maximum(ages[:, None] @ inputs["age_w1"] + inputs["age_b1"], 0.0)
    af = (af @ inputs["age_w2"] + inputs["age_b2"]).astype(np.float32)  # [B, 16]
    A = np.stack([ages, np.ones_like(ages)], axis=1)                    # [B, 2]
    sol, *_ = np.linalg.lstsq(A.astype(np.float64), af.astype(np.float64),
                              rcond=None)
    resid = np.abs(af - (A @ sol.astype(np.float32))).max()
    rank2 = bool(resid <= 1e-4 * max(1.0, np.abs(af).max()))
    c_age, c0_age = sol[0].astype(np.float32), sol[1].astype(np.float32)

    key = ("nc", rank2)
    if key not in _CACHE:
        _CACHE[key] = build_program(rank2)
    nc = _CACHE[key]

    w = _prep_weights(
        inputs["bn_w1"], inputs["bn_b1"], inputs["bn_w2"], inputs["bn_b2"],
        inputs["gm_w1"], inputs["gm_b1"], inputs["gm_w2"], inputs["gm_b2"],
        inputs["fc_w1"], inputs["fc_b1"], inputs["fc_w2"], inputs["fc_b2"],
        c_age, c0_age, rank2)

    gsT_full = np.ascontiguousarray(gs.transpose(1, 2, 0).astype(NP_MM))  # [S, D, B]
    ages16 = ages.astype(NP_MM)
    afT_full = np.ascontiguousarray(af.T.astype(NP_MM))                   # [16, B]
    in_maps = []
    for c in range(N_CORES):
        sl = slice(c * BC, (c + 1) * BC)
        m = dict(w)
        # tile-contiguous layouts: one DMA per tile = 128 fat descriptors
        # (strided gs DMAs cost 1.4-2.4us of issue time each -- measured)
        gsc = gsT_full[:, :, sl].reshape(S, 4, 128, N_CHUNKS, NB)
        m["gsTd"] = np.ascontiguousarray(
            gsc.transpose(0, 3, 2, 1, 4).reshape(S, N_CHUNKS, 128, 4 * NB))
        m["gs8Td"] = np.ascontiguousarray(
            gsc.transpose(3, 2, 0, 1, 4).reshape(N_CHUNKS, 128, S * 4 * NB)
            .astype(NP_F8))
        if rank2:
            m["ageB"] = np.ascontiguousarray(
                np.broadcast_to(ages16[None, sl], (128, BC)))
        else:
            m["afT"] = np.ascontiguousarray(afT_full[:, sl])
        in_maps.append(m)

    res = run_bass_kernel_spmd(nc, in_maps, core_ids=list(range(N_CORES)),
                               trace=trace)
    yT = np.concatenate([res.results[c]["yT"][:, :, :] for c in range(N_CORES)],
                        axis=2)                              # [S, D, B] fp16
    y = yT.astype(np.float32).transpose(2, 0, 1)             # [B, S, D]
    return np.ascontiguousarray(y), res


def kernel(**inputs) -> np.ndarray:
    y, _ = run(inputs, trace=False)
    return y
